# revision 24
# baseline (speedup 1.0000x reference)
"""Bahdanau attention on 8 Trainium2 NeuronCores (Bass/Tile).

Problem (per reference):
  decoder_hidden (64, 512) f32, encoder_outputs (4096, 64, 512) f32,
  W1 (512,512), W2 (512,512), v (512,)
  dec_proj = decoder_hidden @ W1.T                       (B, H)
  enc_proj = einsum('bsh,gh->bsg', enc, W2)              (B, S, H)
  energy   = tanh(dec_proj[:,None,:] + enc_proj) @ v     (B, S)
  attn     = softmax(energy, axis=1)                     (B, S)
  context  = einsum('bs,bsh->bh', attn, enc)             (B, H)
  returns (context, attn)

Sharding: batch (64) split across 8 cores -> 8 batches/core; W1/W2/v
replicated. encoder_outputs is resharded host-side to (b, h, s) layout per
core so the contraction dim h lands on SBUF partitions; the kernel makes a
single pass over the 64 MB/core stream.

Per 512-column s-tile (one DMA of [128p, 4hc, 512s] f32, 2 KB rows):
  cast f32->bf16 (DVE)
  PT[g,s] = W2T-chunk.T @ encT-chunk   16 matmuls into one 4-bank PSUM tile
  tanh(PT + dec_projT[g,b])            4 ACT ops, bias folded per-partition
  energy = v.T @ tanhPT                4 matmuls -> psum [1, 512]
  w = exp(energy) (+ running sum via ACT accum), cast w bf16 (DVE)
  broadcast w across partitions (GPSIMD)
  ctx partials += encT * w             4 DVE scalar_tensor_tensor accums
End of batch: reduce partials, softmax-normalize, DMA outputs.
"""

import numpy as np
import ml_dtypes
from collections import deque

import concourse.bacc as bacc
import concourse.tile as tile
import concourse.mybir as mybir
import concourse.bass_isa as bass_isa
from concourse.bass_utils import run_bass_kernel_spmd

F32 = mybir.dt.float32
BF16 = mybir.dt.bfloat16
AF = mybir.ActivationFunctionType

NB = 8         # batches per core
H = 512
P = 128        # partitions
NCH = H // P   # h chunks (4)
TS = 512       # s columns per tile

TRACE = False
LAST_RESULTS = None


def build(nc, s_len):
    nt = s_len // TS  # s tiles per batch

    enc_d = nc.dram_tensor("enc", [NB, H, s_len], F32, kind="ExternalInput")
    dect_d = nc.dram_tensor("dect", [H, NB], F32, kind="ExternalInput")
    w1t_d = nc.dram_tensor("w1t", [H, H], F32, kind="ExternalInput")
    w2t_d = nc.dram_tensor("w2t", [H, H], F32, kind="ExternalInput")
    v_d = nc.dram_tensor("v2d", [1, H], F32, kind="ExternalInput")
    ctx_d = nc.dram_tensor("ctx_out", [NB, H], F32, kind="ExternalOutput")
    attn_d = nc.dram_tensor("attn_out", [NB, s_len], F32, kind="ExternalOutput")

    ones_d = nc.inline_tensor(np.ones((P, P), dtype=ml_dtypes.bfloat16), name="onespp")
    nt_ = s_len // TS
    G_ = min(4, nt_)
    mask_np = np.zeros((P, 1), dtype=np.float32)
    for j in range(G_):
        mask_np[32 * j, 0] = 1.0
    mask_d = nc.inline_tensor(mask_np, name="maskg")

    # persistent SBUF
    w2t_bf = nc.alloc_sbuf_tensor("w2t_bf", [P, NCH, H], BF16)   # [h, hc, g]
    ones_sb = nc.alloc_sbuf_tensor("ones_sb", [P, P], BF16)
    mask_sb = nc.alloc_sbuf_tensor("mask_sb", [P, 1], F32)
    v_sb = nc.alloc_sbuf_tensor("v_sb", [P, NCH], BF16)          # v chunks [g, gc]
    v32_sb = nc.alloc_sbuf_tensor("v32_sb", [P, NCH, 32], BF16)  # v padded for col-pack
    dpt_sb = nc.alloc_sbuf_tensor("dpt_sb", [P, NCH, NB], F32)   # dec_projT [g, gc, b]

    with tile.TileContext(nc) as tc:
        # ---------------- prologue ----------------
        with (
            tc.tile_pool(name="pro", bufs=1) as pro,
            tc.tile_pool(name="prop", bufs=1, space="PSUM") as prop,
        ):
            w2t_f = pro.tile([P, NCH, H], F32)
            nc.sync.dma_start(out=w2t_f[:], in_=w2t_d.ap().rearrange("(c p) g -> p c g", p=P))
            nc.scalar.copy(w2t_bf[:], w2t_f[:])

            nc.sync.dma_start(out=ones_sb[:], in_=ones_d[:])
            nc.sync.dma_start(out=mask_sb[:], in_=mask_d[:])

            v_f = pro.tile([P, NCH], F32)
            # v[g] -> [g % 128, g // 128]
            nc.sync.dma_start(out=v_f[:], in_=v_d.ap().rearrange("o (c p) -> p (o c)", p=P))
            nc.vector.tensor_copy(v_sb[:], v_f[:])
            nc.vector.memset(v32_sb[:], 0.0)
            for gc in range(NCH):
                nc.vector.tensor_copy(v32_sb[:, gc, 0:1], v_sb[:, gc : gc + 1])

            w1t_f = pro.tile([P, NCH, H], F32)
            nc.sync.dma_start(out=w1t_f[:], in_=w1t_d.ap().rearrange("(c p) g -> p c g", p=P))
            w1t_bf = pro.tile([P, NCH, H], BF16)
            nc.scalar.copy(w1t_bf[:], w1t_f[:])

            dect_f = pro.tile([P, NCH, NB], F32)
            nc.sync.dma_start(out=dect_f[:], in_=dect_d.ap().rearrange("(c p) b -> p c b", p=P))
            dect_bf = pro.tile([P, NCH, NB], BF16)
            nc.scalar.copy(dect_bf[:], dect_f[:])

            # dec_projT[g, b] = sum_h2 W1[g, h2] dec[b, h2]
            dp_ps = prop.tile([P, NCH, NB], F32)
            for gc in range(NCH):
                for hc in range(NCH):
                    nc.tensor.matmul(
                        dp_ps[:, gc, :],
                        w1t_bf[:, hc, gc * P : (gc + 1) * P],
                        dect_bf[:, hc, :],
                        start=(hc == 0), stop=(hc == NCH - 1),
                    )
            nc.scalar.copy(dpt_sb[:], dp_ps[:])

        # ---------------- main loop (software-pipelined emission) ----------------
        enc_r = [enc_d[b].rearrange("(c p) s -> p c s", p=P) for b in range(NB)]
        G = min(4, nt)  # energy col-pack group size
        assert nt % G == 0
        total = NB * nt

        with (
            tc.tile_pool(name="pio", bufs=7) as pio,
            tc.tile_pool(name="pbf", bufs=14) as pbf,
            tc.tile_pool(name="ptan", bufs=10) as ptan,
            tc.tile_pool(name="pw4", bufs=4) as pw4,
            tc.tile_pool(name="pacc", bufs=4) as pacc,
            tc.tile_pool(name="pwb", bufs=3) as pwb,
            tc.tile_pool(name="ppart", bufs=2) as ppart,
            tc.tile_pool(name="psml", bufs=2) as psml,
            tc.tile_pool(name="ppP", bufs=1, space="PSUM") as ppP,
            tc.tile_pool(name="ppe", bufs=1, space="PSUM") as ppe,
            tc.tile_pool(name="ppsum", bufs=1, space="PSUM") as ppsum,
            tc.tile_pool(name="ppb", bufs=2, space="PSUM") as ppb,
        ):
            pending = deque()
            tfs = {}      # k -> f32 enc tile
            tbs = {}      # k -> bf16 enc tile
            tanhs = {}    # k -> tanh tile
            part_ts = {}  # b -> ctx partials
            w4s = {}      # group -> exp(energy) rows [128, TS] (rows 32j)
            accws = {}    # group -> per-row sums [128, 1] (rows 32j)

            def loadcast(k):
                """DMA + cast for tile k (keeps DVE casts ahead of ctx blocks)."""
                b, t = divmod(k, nt)
                if t == 0:
                    part_ts[b] = ppart.tile([P, NCH, nt], F32, tag="part", name="part")
                tf = pio.tile([P, NCH, TS], F32)
                nc.sync.dma_start(out=tf[:], in_=enc_r[b][:, :, t * TS : (t + 1) * TS])
                tb = pbf.tile([P, NCH, TS], BF16)
                nc.vector.tensor_copy(tb[:], tf[:])
                tfs[k] = tf
                tbs[k] = tb

            def pmmtanh(k):
                b, t = divmod(k, nt)
                tb = tbs[k]
                pt_ps = ppP.tile([P, NCH, TS], F32)
                for gc in range(NCH):
                    for hc in range(NCH):
                        nc.tensor.matmul(
                            pt_ps[:, gc, :],
                            w2t_bf[:, hc, gc * P : (gc + 1) * P],
                            tb[:, hc, :],
                            start=(hc == 0), stop=(hc == NCH - 1),
                        )
                tanh_t = ptan.tile([P, NCH, TS], BF16)
                for gc in range(NCH):
                    nc.scalar.activation(
                        tanh_t[:, gc, :], pt_ps[:, gc, :], AF.Tanh,
                        bias=dpt_sb[:, gc, b : b + 1],
                    )
                tanhs[k] = tanh_t

            def process_energy(g):
                """Col-packed energy matmuls + one packed exp + PE broadcast +
                ctx accumulation for tiles Gg..Gg+G-1 (all same batch)."""
                b = (g * G) // nt
                e4 = ppe.tile([P, TS], F32, tag="e4", name="e4")
                for gc in range(NCH):
                    for j in range(G):
                        k = g * G + j
                        nc.tensor.matmul(
                            e4[32 * j : 32 * j + 32, :],
                            v32_sb[:, gc, :], tanhs[k][:, gc, :],
                            start=(gc == 0), stop=(gc == NCH - 1),
                            tile_position=(0, 32 * j),
                            skip_group_check=True,
                        )
                # one exp for the whole group; per-partition accum gives the
                # softmax partial sums on rows 32j for free
                w4 = pw4.tile([P, TS], F32, tag="w4", name="w4")
                accw = pacc.tile([P, 1], F32, tag="accw", name="accw")
                GG = 32 * G
                nc.scalar.activation(w4[0:GG, :], e4[0:GG, :], AF.Exp,
                                     accum_out=accw[0:GG, :])
                w4b = pw4.tile([P, TS], BF16, tag="w4b", name="w4b")
                nc.vector.tensor_copy(w4b[0:GG, :], w4[0:GG, :])
                w4s[g] = w4
                accws[g] = accw

                for j in range(G):
                    pending.append((g * G + j, w4b, j))

            def endb(b):
                part_t = part_ts.pop(b)
                gpb = nt // G  # groups per batch
                g0 = b * gpb
                GG = 32 * G
                acc = accws[g0]
                if gpb > 1:
                    acc_t = psml.tile([P, 1], F32, tag="acct")
                    nc.vector.tensor_tensor(
                        out=acc_t[0:GG, :], in0=accws[g0][0:GG, :],
                        in1=accws[g0 + 1][0:GG, :], op=mybir.AluOpType.add)
                    for g in range(2, gpb):
                        nc.vector.tensor_tensor(
                            out=acc_t[0:GG, :], in0=acc_t[0:GG, :],
                            in1=accws[g0 + g][0:GG, :], op=mybir.AluOpType.add)
                    acc = acc_t
                # sum of rows {32j} only via masked fp32 matmul
                sum_ps = ppsum.tile([1, 1], F32, tag="sum", name="sum")
                nc.tensor.matmul(sum_ps[:], acc[0:GG, :], mask_sb[0:GG, :],
                                 start=True, stop=True)
                inv1 = psml.tile([1, 1], F32, tag="inv1")
                nc.vector.reciprocal(inv1[:], sum_ps[:])
                inv_b = psml.tile([P, 1], F32, tag="invb")
                nc.gpsimd.partition_broadcast(inv_b[:], inv1[:])

                for g in range(g0, g0 + gpb):
                    w4 = w4s.pop(g)
                    nc.vector.tensor_scalar(
                        out=w4[0:GG, :], in0=w4[0:GG, :],
                        scalar1=inv_b[0:GG, :], scalar2=None,
                        op0=mybir.AluOpType.mult,
                    )
                    for j in range(G):
                        s0 = (g * G + j) % nt * TS
                        nc.sync.dma_start(
                            out=attn_d[b : b + 1, s0 : s0 + TS],
                            in_=w4[32 * j : 32 * j + 1, :])
                    accws.pop(g, None)

                ctx_red = psml.tile([P, NCH], F32, tag="ctxred")
                nc.vector.reduce_sum(ctx_red[:], part_t[:], axis=mybir.AxisListType.X)
                ctx_sb = psml.tile([P, NCH], F32, tag="ctxo")
                nc.vector.tensor_scalar(
                    out=ctx_sb[:], in0=ctx_red[:], scalar1=inv_b[:],
                    scalar2=None, op0=mybir.AluOpType.mult,
                )
                nc.sync.dma_start(out=ctx_d[b].rearrange("(c p) -> p c", p=P), in_=ctx_sb[:])

            def do_pending(n):
                """Emit wb broadcast-MM + ctx stt block for up to n pending
                tiles (spread across iterations so PE never stalls on the
                wb PSUM WAR against in-flight ctx blocks)."""
                for _ in range(min(n, len(pending))):
                    k, w4b, j = pending.popleft()
                    b, t = divmod(k, nt)
                    wb_ps = ppb.tile([P, TS], F32, tag="wb", name="wb")
                    nc.tensor.matmul(
                        wb_ps[:], ones_sb[32 * j : 32 * j + 1, :],
                        w4b[32 * j : 32 * j + 1, :],
                        start=True, stop=True, tile_position=(32 * j, 0),
                    )
                    wb_sb = pwb.tile([P, TS], BF16, tag="wbsb", name="wbsb")
                    nc.scalar.copy(wb_sb[:], wb_ps[:])
                    tb = tbs.pop(k)
                    for hc in range(NCH):
                        nc.vector.scalar_tensor_tensor(
                            out=tb[:, hc, :], in0=tb[:, hc, :], scalar=1.0,
                            in1=wb_sb[:],
                            op0=mybir.AluOpType.mult, op1=mybir.AluOpType.mult,
                            accum_out=part_ts[b][:, hc, t : t + 1],
                        )
                    tfs.pop(k, None)
                    tanhs.pop(k, None)
                    if t == nt - 1:
                        endb(b)

            # pipeline: loads/casts run PL tiles ahead; energy group g is
            # emitted after the P-matmuls of group g+1; ctx blocks trail one
            # tile per iteration so no engine stalls behind them.
            ngroups = total // G
            PL = 4
            for k in range(total + PL):
                if k < total:
                    loadcast(k)
                kp = k - PL
                if kp >= 0:
                    pmmtanh(kp)
                    do_pending(1)
                    if kp % G == G - 1 and kp // G >= 1:
                        process_energy(kp // G - 1)
            process_energy(ngroups - 1)
            do_pending(len(pending) + G)

    return nc


_CACHE = {}


def _get_nc(s_len):
    if s_len not in _CACHE:
        nc = bacc.Bacc("TRN2", target_bir_lowering=False, debug=False)
        build(nc, s_len)
        nc.compile()
        _CACHE[s_len] = nc
    return _CACHE[s_len]


def _prep_inputs(decoder_hidden, encoder_outputs, W1, W2, v):
    """Host-side shard: batch across 8 cores; encT layout (b, h, s) per core."""
    s_len = encoder_outputs.shape[0]
    w1t = np.ascontiguousarray(np.asarray(W1, dtype=np.float32).T)
    w2t = np.ascontiguousarray(np.asarray(W2, dtype=np.float32).T)
    v2d = np.ascontiguousarray(np.asarray(v, dtype=np.float32).reshape(1, H))
    enc = np.asarray(encoder_outputs, dtype=np.float32)
    dec = np.asarray(decoder_hidden, dtype=np.float32)
    in_maps = []
    for c in range(8):
        bsl = slice(c * NB, (c + 1) * NB)
        enc_c = np.ascontiguousarray(enc[:, bsl, :].transpose(1, 2, 0))
        dect_c = np.ascontiguousarray(dec[bsl, :].T)
        in_maps.append(
            {"enc": enc_c, "dect": dect_c, "w1t": w1t, "w2t": w2t, "v2d": v2d}
        )
    return in_maps, s_len


def kernel(decoder_hidden, encoder_outputs, W1, W2, v):
    global LAST_RESULTS
    in_maps, s_len = _prep_inputs(decoder_hidden, encoder_outputs, W1, W2, v)
    nc = _get_nc(s_len)
    res = run_bass_kernel_spmd(nc, in_maps, core_ids=list(range(8)), trace=TRACE)
    LAST_RESULTS = res
    B = 8 * NB
    context = np.empty((B, H), dtype=np.float32)
    attn = np.empty((B, s_len), dtype=np.float32)
    for c in range(8):
        bsl = slice(c * NB, (c + 1) * NB)
        context[bsl] = res.results[c]["ctx_out"]
        attn[bsl] = res.results[c]["attn_out"]
    return (context, attn)


# revision 25
# speedup vs baseline: 1.0475x; 1.0475x over previous
"""Bahdanau attention on 8 Trainium2 NeuronCores (Bass/Tile).

Problem (per reference):
  decoder_hidden (64, 512) f32, encoder_outputs (4096, 64, 512) f32,
  W1 (512,512), W2 (512,512), v (512,)
  dec_proj = decoder_hidden @ W1.T                       (B, H)
  enc_proj = einsum('bsh,gh->bsg', enc, W2)              (B, S, H)
  energy   = tanh(dec_proj[:,None,:] + enc_proj) @ v     (B, S)
  attn     = softmax(energy, axis=1)                     (B, S)
  context  = einsum('bs,bsh->bh', attn, enc)             (B, H)
  returns (context, attn)

Sharding: batch (64) split across 8 cores -> 8 batches/core; W1/W2/v
replicated. encoder_outputs is resharded host-side to (b, h, s) layout per
core so the contraction dim h lands on SBUF partitions; the kernel makes a
single pass over the 64 MB/core stream.

Per 512-column s-tile (one DMA of [128p, 4hc, 512s] f32, 2 KB rows):
  cast f32->bf16 (DVE)
  PT[g,s] = W2T-chunk.T @ encT-chunk   16 matmuls into one 4-bank PSUM tile
  tanh(PT + dec_projT[g,b])            4 ACT ops, bias folded per-partition
  energy = v.T @ tanhPT                4 matmuls -> psum [1, 512]
  w = exp(energy) (+ running sum via ACT accum), cast w bf16 (DVE)
  broadcast w across partitions (GPSIMD)
  ctx partials += encT * w             4 DVE scalar_tensor_tensor accums
End of batch: reduce partials, softmax-normalize, DMA outputs.
"""

import numpy as np
import ml_dtypes
from collections import deque

import concourse.bacc as bacc
import concourse.tile as tile
import concourse.mybir as mybir
import concourse.bass_isa as bass_isa
from concourse.bass_utils import run_bass_kernel_spmd

F32 = mybir.dt.float32
BF16 = mybir.dt.bfloat16
AF = mybir.ActivationFunctionType

NB = 8         # batches per core
H = 512
P = 128        # partitions
NCH = H // P   # h chunks (4)
TS = 512       # s columns per tile

TRACE = False
LAST_RESULTS = None


def build(nc, s_len):
    nt = s_len // TS  # s tiles per batch

    enc_d = nc.dram_tensor("enc", [NB, H, s_len], F32, kind="ExternalInput")
    dect_d = nc.dram_tensor("dect", [H, NB], F32, kind="ExternalInput")
    w1t_d = nc.dram_tensor("w1t", [H, H], F32, kind="ExternalInput")
    w2t_d = nc.dram_tensor("w2t", [H, H], F32, kind="ExternalInput")
    v_d = nc.dram_tensor("v2d", [1, H], F32, kind="ExternalInput")
    ctx_d = nc.dram_tensor("ctx_out", [NB, H], F32, kind="ExternalOutput")
    attn_d = nc.dram_tensor("attn_out", [NB, s_len], F32, kind="ExternalOutput")

    ones_d = nc.inline_tensor(np.ones((P, P), dtype=ml_dtypes.bfloat16), name="onespp")
    nt_ = s_len // TS
    G_ = min(4, nt_)
    mask_np = np.zeros((P, 1), dtype=np.float32)
    for j in range(G_):
        mask_np[32 * j, 0] = 1.0
    mask_d = nc.inline_tensor(mask_np, name="maskg")

    # persistent SBUF
    w2t_bf = nc.alloc_sbuf_tensor("w2t_bf", [P, NCH, H], BF16)   # [h, hc, g]
    ones_sb = nc.alloc_sbuf_tensor("ones_sb", [P, P], BF16)
    mask_sb = nc.alloc_sbuf_tensor("mask_sb", [P, 1], F32)
    v_sb = nc.alloc_sbuf_tensor("v_sb", [P, NCH], BF16)          # v chunks [g, gc]
    v32_sb = nc.alloc_sbuf_tensor("v32_sb", [P, NCH, 32], BF16)  # v padded for col-pack
    dpt_sb = nc.alloc_sbuf_tensor("dpt_sb", [P, NCH, NB], F32)   # dec_projT [g, gc, b]

    with tile.TileContext(nc) as tc:
        # ---------------- prologue ----------------
        with (
            tc.tile_pool(name="pro", bufs=1) as pro,
            tc.tile_pool(name="prop", bufs=1, space="PSUM") as prop,
        ):
            w2t_f = pro.tile([P, NCH, H], F32)
            nc.sync.dma_start(out=w2t_f[:], in_=w2t_d.ap().rearrange("(c p) g -> p c g", p=P))
            nc.scalar.copy(w2t_bf[:], w2t_f[:])

            nc.sync.dma_start(out=ones_sb[:], in_=ones_d[:])
            nc.sync.dma_start(out=mask_sb[:], in_=mask_d[:])

            v_f = pro.tile([P, NCH], F32)
            # v[g] -> [g % 128, g // 128]
            nc.sync.dma_start(out=v_f[:], in_=v_d.ap().rearrange("o (c p) -> p (o c)", p=P))
            nc.vector.tensor_copy(v_sb[:], v_f[:])
            nc.vector.memset(v32_sb[:], 0.0)
            for gc in range(NCH):
                nc.vector.tensor_copy(v32_sb[:, gc, 0:1], v_sb[:, gc : gc + 1])

            w1t_f = pro.tile([P, NCH, H], F32)
            nc.sync.dma_start(out=w1t_f[:], in_=w1t_d.ap().rearrange("(c p) g -> p c g", p=P))
            w1t_bf = pro.tile([P, NCH, H], BF16)
            nc.scalar.copy(w1t_bf[:], w1t_f[:])

            dect_f = pro.tile([P, NCH, NB], F32)
            nc.sync.dma_start(out=dect_f[:], in_=dect_d.ap().rearrange("(c p) b -> p c b", p=P))
            dect_bf = pro.tile([P, NCH, NB], BF16)
            nc.scalar.copy(dect_bf[:], dect_f[:])

            # dec_projT[g, b] = sum_h2 W1[g, h2] dec[b, h2]
            dp_ps = prop.tile([P, NCH, NB], F32)
            for gc in range(NCH):
                for hc in range(NCH):
                    nc.tensor.matmul(
                        dp_ps[:, gc, :],
                        w1t_bf[:, hc, gc * P : (gc + 1) * P],
                        dect_bf[:, hc, :],
                        start=(hc == 0), stop=(hc == NCH - 1),
                    )
            nc.scalar.copy(dpt_sb[:], dp_ps[:])

        # ---------------- main loop (software-pipelined emission) ----------------
        enc_r = [enc_d[b].rearrange("(c p) s -> p c s", p=P) for b in range(NB)]
        G = min(4, nt)  # energy col-pack group size
        assert nt % G == 0
        total = NB * nt

        with (
            tc.tile_pool(name="pio", bufs=7) as pio,
            tc.tile_pool(name="pbf", bufs=14) as pbf,
            tc.tile_pool(name="ptan", bufs=10) as ptan,
            tc.tile_pool(name="pw4", bufs=4) as pw4,
            tc.tile_pool(name="pacc", bufs=4) as pacc,
            tc.tile_pool(name="ppart", bufs=2) as ppart,
            tc.tile_pool(name="psml", bufs=2) as psml,
            tc.tile_pool(name="ppP", bufs=1, space="PSUM") as ppP,
            tc.tile_pool(name="ppe", bufs=1, space="PSUM") as ppe,
            tc.tile_pool(name="ppsum", bufs=1, space="PSUM") as ppsum,
            tc.tile_pool(name="ppb", bufs=2, space="PSUM") as ppb,
        ):
            pending = deque()
            tfs = {}      # k -> f32 enc tile
            tbs = {}      # k -> bf16 enc tile
            tanhs = {}    # k -> tanh tile
            part_ts = {}  # b -> ctx partials
            w4s = {}      # group -> exp(energy) rows [128, TS] (rows 32j)
            accws = {}    # group -> per-row sums [128, 1] (rows 32j)

            def loadcast(k):
                """DMA + cast for tile k (keeps DVE casts ahead of ctx blocks)."""
                b, t = divmod(k, nt)
                if t == 0:
                    part_ts[b] = ppart.tile([P, NCH, nt], F32, tag="part", name="part")
                tf = pio.tile([P, NCH, TS], F32)
                nc.sync.dma_start(out=tf[:], in_=enc_r[b][:, :, t * TS : (t + 1) * TS])
                tb = pbf.tile([P, NCH, TS], BF16)
                nc.vector.tensor_copy(tb[:], tf[:])
                tfs[k] = tf
                tbs[k] = tb

            def pmmtanh(k):
                b, t = divmod(k, nt)
                tb = tbs[k]
                pt_ps = ppP.tile([P, NCH, TS], F32)
                for gc in range(NCH):
                    for hc in range(NCH):
                        nc.tensor.matmul(
                            pt_ps[:, gc, :],
                            w2t_bf[:, hc, gc * P : (gc + 1) * P],
                            tb[:, hc, :],
                            start=(hc == 0), stop=(hc == NCH - 1),
                        )
                tanh_t = ptan.tile([P, NCH, TS], BF16)
                for gc in range(NCH):
                    nc.scalar.activation(
                        tanh_t[:, gc, :], pt_ps[:, gc, :], AF.Tanh,
                        bias=dpt_sb[:, gc, b : b + 1],
                    )
                tanhs[k] = tanh_t

            def process_energy(g):
                """Col-packed energy matmuls + one packed exp + PE broadcast +
                ctx accumulation for tiles Gg..Gg+G-1 (all same batch)."""
                b = (g * G) // nt
                e4 = ppe.tile([P, TS], F32, tag="e4", name="e4")
                for gc in range(NCH):
                    for j in range(G):
                        k = g * G + j
                        nc.tensor.matmul(
                            e4[32 * j : 32 * j + 32, :],
                            v32_sb[:, gc, :], tanhs[k][:, gc, :],
                            start=(gc == 0), stop=(gc == NCH - 1),
                            tile_position=(0, 32 * j),
                            skip_group_check=True,
                        )
                # one exp for the whole group; per-partition accum gives the
                # softmax partial sums on rows 32j for free
                w4 = pw4.tile([P, TS], F32, tag="w4", name="w4")
                accw = pacc.tile([P, 1], F32, tag="accw", name="accw")
                GG = 32 * G
                nc.scalar.activation(w4[0:GG, :], e4[0:GG, :], AF.Exp,
                                     accum_out=accw[0:GG, :])
                w4b = pw4.tile([P, TS], BF16, tag="w4b", name="w4b")
                nc.vector.tensor_copy(w4b[0:GG, :], w4[0:GG, :])
                w4s[g] = w4
                accws[g] = accw

                for j in range(G):
                    pending.append((g * G + j, w4b, j))

            def endb(b):
                part_t = part_ts.pop(b)
                gpb = nt // G  # groups per batch
                g0 = b * gpb
                GG = 32 * G
                acc = accws[g0]
                if gpb > 1:
                    acc_t = psml.tile([P, 1], F32, tag="acct")
                    nc.vector.tensor_tensor(
                        out=acc_t[0:GG, :], in0=accws[g0][0:GG, :],
                        in1=accws[g0 + 1][0:GG, :], op=mybir.AluOpType.add)
                    for g in range(2, gpb):
                        nc.vector.tensor_tensor(
                            out=acc_t[0:GG, :], in0=acc_t[0:GG, :],
                            in1=accws[g0 + g][0:GG, :], op=mybir.AluOpType.add)
                    acc = acc_t
                # sum of rows {32j} only via masked fp32 matmul
                sum_ps = ppsum.tile([1, 1], F32, tag="sum", name="sum")
                nc.tensor.matmul(sum_ps[:], acc[0:GG, :], mask_sb[0:GG, :],
                                 start=True, stop=True)
                inv1 = psml.tile([1, 1], F32, tag="inv1")
                nc.vector.reciprocal(inv1[:], sum_ps[:])
                inv_b = psml.tile([P, 1], F32, tag="invb")
                nc.gpsimd.partition_broadcast(inv_b[:], inv1[:])

                for g in range(g0, g0 + gpb):
                    w4 = w4s.pop(g)
                    nc.vector.tensor_scalar(
                        out=w4[0:GG, :], in0=w4[0:GG, :],
                        scalar1=inv_b[0:GG, :], scalar2=None,
                        op0=mybir.AluOpType.mult,
                    )
                    for j in range(G):
                        s0 = (g * G + j) % nt * TS
                        nc.sync.dma_start(
                            out=attn_d[b : b + 1, s0 : s0 + TS],
                            in_=w4[32 * j : 32 * j + 1, :])
                    accws.pop(g, None)

                ctx_red = psml.tile([P, NCH], F32, tag="ctxred")
                nc.vector.reduce_sum(ctx_red[:], part_t[:], axis=mybir.AxisListType.X)
                ctx_sb = psml.tile([P, NCH], F32, tag="ctxo")
                nc.vector.tensor_scalar(
                    out=ctx_sb[:], in0=ctx_red[:], scalar1=inv_b[:],
                    scalar2=None, op0=mybir.AluOpType.mult,
                )
                nc.sync.dma_start(out=ctx_d[b].rearrange("(c p) -> p c", p=P), in_=ctx_sb[:])

            def do_pending(n):
                """Emit wb broadcast-MM + ctx stt block for up to n pending
                tiles (spread across iterations so PE never stalls on the
                wb PSUM WAR against in-flight ctx blocks)."""
                for _ in range(min(n, len(pending))):
                    k, w4b, j = pending.popleft()
                    b, t = divmod(k, nt)
                    wb_ps = ppb.tile([P, TS], F32, tag="wb", name="wb")
                    nc.tensor.matmul(
                        wb_ps[:], ones_sb[32 * j : 32 * j + 1, :],
                        w4b[32 * j : 32 * j + 1, :],
                        start=True, stop=True, tile_position=(32 * j, 0),
                    )
                    tb = tbs.pop(k)
                    for hc in range(NCH):
                        nc.vector.scalar_tensor_tensor(
                            out=tb[:, hc, :], in0=tb[:, hc, :], scalar=1.0,
                            in1=wb_ps[:],
                            op0=mybir.AluOpType.mult, op1=mybir.AluOpType.mult,
                            accum_out=part_ts[b][:, hc, t : t + 1],
                        )
                    tfs.pop(k, None)
                    tanhs.pop(k, None)
                    if t == nt - 1:
                        endb(b)

            # pipeline: loads/casts run PL tiles ahead; energy group g is
            # emitted after the P-matmuls of group g+1; ctx blocks trail one
            # tile per iteration so no engine stalls behind them.
            ngroups = total // G
            PL = 4
            for k in range(total + PL):
                if k < total:
                    loadcast(k)
                kp = k - PL
                if kp >= 0:
                    pmmtanh(kp)
                    do_pending(1)
                    if kp % G == G - 1 and kp // G >= 1:
                        process_energy(kp // G - 1)
            process_energy(ngroups - 1)
            do_pending(len(pending) + G)

    return nc


_CACHE = {}


def _get_nc(s_len):
    if s_len not in _CACHE:
        nc = bacc.Bacc("TRN2", target_bir_lowering=False, debug=False)
        build(nc, s_len)
        nc.compile()
        _CACHE[s_len] = nc
    return _CACHE[s_len]


def _prep_inputs(decoder_hidden, encoder_outputs, W1, W2, v):
    """Host-side shard: batch across 8 cores; encT layout (b, h, s) per core."""
    s_len = encoder_outputs.shape[0]
    w1t = np.ascontiguousarray(np.asarray(W1, dtype=np.float32).T)
    w2t = np.ascontiguousarray(np.asarray(W2, dtype=np.float32).T)
    v2d = np.ascontiguousarray(np.asarray(v, dtype=np.float32).reshape(1, H))
    enc = np.asarray(encoder_outputs, dtype=np.float32)
    dec = np.asarray(decoder_hidden, dtype=np.float32)
    in_maps = []
    for c in range(8):
        bsl = slice(c * NB, (c + 1) * NB)
        enc_c = np.ascontiguousarray(enc[:, bsl, :].transpose(1, 2, 0))
        dect_c = np.ascontiguousarray(dec[bsl, :].T)
        in_maps.append(
            {"enc": enc_c, "dect": dect_c, "w1t": w1t, "w2t": w2t, "v2d": v2d}
        )
    return in_maps, s_len


def kernel(decoder_hidden, encoder_outputs, W1, W2, v):
    global LAST_RESULTS
    in_maps, s_len = _prep_inputs(decoder_hidden, encoder_outputs, W1, W2, v)
    nc = _get_nc(s_len)
    res = run_bass_kernel_spmd(nc, in_maps, core_ids=list(range(8)), trace=TRACE)
    LAST_RESULTS = res
    B = 8 * NB
    context = np.empty((B, H), dtype=np.float32)
    attn = np.empty((B, s_len), dtype=np.float32)
    for c in range(8):
        bsl = slice(c * NB, (c + 1) * NB)
        context[bsl] = res.results[c]["ctx_out"]
        attn[bsl] = res.results[c]["attn_out"]
    return (context, attn)


# revision 26
# speedup vs baseline: 1.5217x; 1.4527x over previous
"""Bahdanau attention on 8 Trainium2 NeuronCores (Bass/Tile).

Problem (per reference):
  decoder_hidden (64, 512) f32, encoder_outputs (4096, 64, 512) f32,
  W1 (512,512), W2 (512,512), v (512,)
  dec_proj = decoder_hidden @ W1.T                       (B, H)
  enc_proj = einsum('bsh,gh->bsg', enc, W2)              (B, S, H)
  energy   = tanh(dec_proj[:,None,:] + enc_proj) @ v     (B, S)
  attn     = softmax(energy, axis=1)                     (B, S)
  context  = einsum('bs,bsh->bh', attn, enc)             (B, H)
  returns (context, attn)

Sharding: batch (64) split across 8 cores -> 8 batches/core; W1/W2/v
replicated. encoder_outputs is resharded host-side to (b, h, s) layout per
core so the contraction dim h lands on SBUF partitions; the kernel makes a
single pass over the 64 MB/core stream.

Per 512-column s-tile (one DMA of [128p, 4hc, 512s] f32, 2 KB rows):
  cast f32->bf16 (DVE)
  PT[g,s] = W2T-chunk.T @ encT-chunk   16 matmuls into one 4-bank PSUM tile
  tanh(PT + dec_projT[g,b])            4 ACT ops, bias folded per-partition
  energy = v.T @ tanhPT                4 matmuls -> psum [1, 512]
  w = exp(energy) (+ running sum via ACT accum), cast w bf16 (DVE)
  broadcast w across partitions (GPSIMD)
  ctx partials += encT * w             4 DVE scalar_tensor_tensor accums
End of batch: reduce partials, softmax-normalize, DMA outputs.
"""

import numpy as np
import ml_dtypes
from collections import deque

import concourse.bacc as bacc
import concourse.tile as tile
import concourse.mybir as mybir
import concourse.bass_isa as bass_isa
from concourse.bass_utils import run_bass_kernel_spmd

F32 = mybir.dt.float32
BF16 = mybir.dt.bfloat16
AF = mybir.ActivationFunctionType

NB = 8         # batches per core
H = 512
P = 128        # partitions
NCH = H // P   # h chunks (4)
TS = 512       # s columns per tile

TRACE = False
LAST_RESULTS = None


def build(nc, s_len):
    nt = s_len // TS  # s tiles per batch

    enc_d = nc.dram_tensor("enc", [NB, H, s_len], F32, kind="ExternalInput")
    dect_d = nc.dram_tensor("dect", [H, NB], F32, kind="ExternalInput")
    w1t_d = nc.dram_tensor("w1t", [H, H], F32, kind="ExternalInput")
    w2t_d = nc.dram_tensor("w2t", [H, H], F32, kind="ExternalInput")
    v_d = nc.dram_tensor("v2d", [1, H], F32, kind="ExternalInput")
    ctx_d = nc.dram_tensor("ctx_out", [NB, H], F32, kind="ExternalOutput")
    attn_d = nc.dram_tensor("attn_out", [NB, s_len], F32, kind="ExternalOutput")

    ones_d = nc.inline_tensor(np.ones((P, P), dtype=ml_dtypes.bfloat16), name="onespp")
    nt_ = s_len // TS
    G_ = min(4, nt_)
    mask_np = np.zeros((P, 1), dtype=np.float32)
    for j in range(G_):
        mask_np[32 * j, 0] = 1.0
    mask_d = nc.inline_tensor(mask_np, name="maskg")

    # persistent SBUF
    w2t_bf = nc.alloc_sbuf_tensor("w2t_bf", [P, NCH, H], BF16)   # [h, hc, g]
    ones_sb = nc.alloc_sbuf_tensor("ones_sb", [P, P], BF16)
    mask_sb = nc.alloc_sbuf_tensor("mask_sb", [P, 1], F32)
    v_sb = nc.alloc_sbuf_tensor("v_sb", [P, NCH], BF16)          # v chunks [g, gc]
    v32_sb = nc.alloc_sbuf_tensor("v32_sb", [P, NCH, 32], BF16)  # v padded for col-pack
    dpt_sb = nc.alloc_sbuf_tensor("dpt_sb", [P, NCH, NB], F32)   # dec_projT [g, gc, b]

    with tile.TileContext(nc) as tc:
        # ---------------- prologue ----------------
        with (
            tc.tile_pool(name="pro", bufs=1) as pro,
            tc.tile_pool(name="prop", bufs=1, space="PSUM") as prop,
        ):
            w2t_f = pro.tile([P, NCH, H], F32)
            nc.sync.dma_start(out=w2t_f[:], in_=w2t_d.ap().rearrange("(c p) g -> p c g", p=P))
            nc.scalar.copy(w2t_bf[:], w2t_f[:])

            nc.sync.dma_start(out=ones_sb[:], in_=ones_d[:])
            nc.sync.dma_start(out=mask_sb[:], in_=mask_d[:])

            v_f = pro.tile([P, NCH], F32)
            # v[g] -> [g % 128, g // 128]
            nc.sync.dma_start(out=v_f[:], in_=v_d.ap().rearrange("o (c p) -> p (o c)", p=P))
            nc.vector.tensor_copy(v_sb[:], v_f[:])
            nc.vector.memset(v32_sb[:], 0.0)
            for gc in range(NCH):
                nc.vector.tensor_copy(v32_sb[:, gc, 0:1], v_sb[:, gc : gc + 1])

            w1t_f = pro.tile([P, NCH, H], F32)
            nc.sync.dma_start(out=w1t_f[:], in_=w1t_d.ap().rearrange("(c p) g -> p c g", p=P))
            w1t_bf = pro.tile([P, NCH, H], BF16)
            nc.scalar.copy(w1t_bf[:], w1t_f[:])

            dect_f = pro.tile([P, NCH, NB], F32)
            nc.sync.dma_start(out=dect_f[:], in_=dect_d.ap().rearrange("(c p) b -> p c b", p=P))
            dect_bf = pro.tile([P, NCH, NB], BF16)
            nc.scalar.copy(dect_bf[:], dect_f[:])

            # dec_projT[g, b] = sum_h2 W1[g, h2] dec[b, h2]
            dp_ps = prop.tile([P, NCH, NB], F32)
            for gc in range(NCH):
                for hc in range(NCH):
                    nc.tensor.matmul(
                        dp_ps[:, gc, :],
                        w1t_bf[:, hc, gc * P : (gc + 1) * P],
                        dect_bf[:, hc, :],
                        start=(hc == 0), stop=(hc == NCH - 1),
                    )
            nc.scalar.copy(dpt_sb[:], dp_ps[:])

        # ---------------- main loop (software-pipelined emission) ----------------
        enc_r = [enc_d[b].rearrange("(c p) s -> p c s", p=P) for b in range(NB)]
        G = min(4, nt)  # energy col-pack group size
        assert nt % G == 0
        total = NB * nt

        with (
            tc.tile_pool(name="pio", bufs=7) as pio,
            tc.tile_pool(name="pbf", bufs=14) as pbf,
            tc.tile_pool(name="ptan", bufs=10) as ptan,
            tc.tile_pool(name="pw4", bufs=4) as pw4,
            tc.tile_pool(name="pacc", bufs=4) as pacc,
            tc.tile_pool(name="ppart", bufs=2) as ppart,
            tc.tile_pool(name="psml", bufs=2) as psml,
            tc.tile_pool(name="ppP", bufs=1, space="PSUM") as ppP,
            tc.tile_pool(name="ppP1", bufs=1, space="PSUM") as ppP1,
            tc.tile_pool(name="ppP2", bufs=1, space="PSUM") as ppP2,
            tc.tile_pool(name="ppP3", bufs=1, space="PSUM") as ppP3,
            tc.tile_pool(name="ppe", bufs=1, space="PSUM") as ppe,
            tc.tile_pool(name="ppsum", bufs=1, space="PSUM") as ppsum,
            tc.tile_pool(name="ppb", bufs=2, space="PSUM") as ppb,
        ):
            pending = deque()
            tfs = {}      # k -> f32 enc tile
            tbs = {}      # k -> bf16 enc tile
            tanhs = {}    # k -> tanh tile
            part_ts = {}  # b -> ctx partials
            w4s = {}      # group -> exp(energy) rows [128, TS] (rows 32j)
            accws = {}    # group -> per-row sums [128, 1] (rows 32j)

            def loadcast(k):
                """DMA + cast for tile k (keeps DVE casts ahead of ctx blocks)."""
                b, t = divmod(k, nt)
                if t == 0:
                    part_ts[b] = ppart.tile([P, NCH, nt], F32, tag="part", name="part")
                tf = pio.tile([P, NCH, TS], F32)
                nc.sync.dma_start(out=tf[:], in_=enc_r[b][:, :, t * TS : (t + 1) * TS])
                tb = pbf.tile([P, NCH, TS], BF16)
                nc.vector.tensor_copy(tb[:], tf[:])
                tfs[k] = tf
                tbs[k] = tb

            def pmmtanh(k):
                b, t = divmod(k, nt)
                tb = tbs[k]
                tanh_t = ptan.tile([P, NCH, TS], BF16)
                for gc in range(NCH):
                    pool = (ppP, ppP1, ppP2, ppP3)[gc]
                    pt_ps = pool.tile([P, TS], F32, tag="pt", name="pt")
                    for hc in range(NCH):
                        nc.tensor.matmul(
                            pt_ps[:],
                            w2t_bf[:, hc, gc * P : (gc + 1) * P],
                            tb[:, hc, :],
                            start=(hc == 0), stop=(hc == NCH - 1),
                        )
                    nc.scalar.activation(
                        tanh_t[:, gc, :], pt_ps[:], AF.Tanh,
                        bias=dpt_sb[:, gc, b : b + 1],
                    )
                tanhs[k] = tanh_t

            def process_energy(g):
                """Col-packed energy matmuls + one packed exp + PE broadcast +
                ctx accumulation for tiles Gg..Gg+G-1 (all same batch)."""
                b = (g * G) // nt
                e4 = ppe.tile([P, TS], F32, tag="e4", name="e4")
                for gc in range(NCH):
                    for j in range(G):
                        k = g * G + j
                        nc.tensor.matmul(
                            e4[32 * j : 32 * j + 32, :],
                            v32_sb[:, gc, :], tanhs[k][:, gc, :],
                            start=(gc == 0), stop=(gc == NCH - 1),
                            tile_position=(0, 32 * j),
                            skip_group_check=True,
                        )
                # one exp for the whole group; per-partition accum gives the
                # softmax partial sums on rows 32j for free
                w4 = pw4.tile([P, TS], F32, tag="w4", name="w4")
                accw = pacc.tile([P, 1], F32, tag="accw", name="accw")
                GG = 32 * G
                nc.scalar.activation(w4[0:GG, :], e4[0:GG, :], AF.Exp,
                                     accum_out=accw[0:GG, :])
                w4b = pw4.tile([P, TS], BF16, tag="w4b", name="w4b")
                nc.vector.tensor_copy(w4b[0:GG, :], w4[0:GG, :])
                w4s[g] = w4
                accws[g] = accw

                for j in range(G):
                    pending.append((g * G + j, w4b, j))

            def endb(b):
                part_t = part_ts.pop(b)
                gpb = nt // G  # groups per batch
                g0 = b * gpb
                GG = 32 * G
                acc = accws[g0]
                if gpb > 1:
                    acc_t = psml.tile([P, 1], F32, tag="acct")
                    nc.vector.tensor_tensor(
                        out=acc_t[0:GG, :], in0=accws[g0][0:GG, :],
                        in1=accws[g0 + 1][0:GG, :], op=mybir.AluOpType.add)
                    for g in range(2, gpb):
                        nc.vector.tensor_tensor(
                            out=acc_t[0:GG, :], in0=acc_t[0:GG, :],
                            in1=accws[g0 + g][0:GG, :], op=mybir.AluOpType.add)
                    acc = acc_t
                # sum of rows {32j} only via masked fp32 matmul
                sum_ps = ppsum.tile([1, 1], F32, tag="sum", name="sum")
                nc.tensor.matmul(sum_ps[:], acc[0:GG, :], mask_sb[0:GG, :],
                                 start=True, stop=True)
                inv1 = psml.tile([1, 1], F32, tag="inv1")
                nc.vector.reciprocal(inv1[:], sum_ps[:])
                inv_b = psml.tile([P, 1], F32, tag="invb")
                nc.gpsimd.partition_broadcast(inv_b[:], inv1[:])

                for g in range(g0, g0 + gpb):
                    w4 = w4s.pop(g)
                    nc.vector.tensor_scalar(
                        out=w4[0:GG, :], in0=w4[0:GG, :],
                        scalar1=inv_b[0:GG, :], scalar2=None,
                        op0=mybir.AluOpType.mult,
                    )
                    for j in range(G):
                        s0 = (g * G + j) % nt * TS
                        nc.sync.dma_start(
                            out=attn_d[b : b + 1, s0 : s0 + TS],
                            in_=w4[32 * j : 32 * j + 1, :])
                    accws.pop(g, None)

                ctx_red = psml.tile([P, NCH], F32, tag="ctxred")
                nc.vector.reduce_sum(ctx_red[:], part_t[:], axis=mybir.AxisListType.X)
                ctx_sb = psml.tile([P, NCH], F32, tag="ctxo")
                nc.vector.tensor_scalar(
                    out=ctx_sb[:], in0=ctx_red[:], scalar1=inv_b[:],
                    scalar2=None, op0=mybir.AluOpType.mult,
                )
                nc.sync.dma_start(out=ctx_d[b].rearrange("(c p) -> p c", p=P), in_=ctx_sb[:])

            def do_pending(n):
                """Emit wb broadcast-MM + ctx stt block for up to n pending
                tiles (spread across iterations so PE never stalls on the
                wb PSUM WAR against in-flight ctx blocks)."""
                for _ in range(min(n, len(pending))):
                    k, w4b, j = pending.popleft()
                    b, t = divmod(k, nt)
                    wb_ps = ppb.tile([P, TS], F32, tag="wb", name="wb")
                    nc.tensor.matmul(
                        wb_ps[:], ones_sb[32 * j : 32 * j + 1, :],
                        w4b[32 * j : 32 * j + 1, :],
                        start=True, stop=True, tile_position=(32 * j, 0),
                    )
                    tb = tbs.pop(k)
                    for hc in range(NCH):
                        nc.vector.scalar_tensor_tensor(
                            out=tb[:, hc, :], in0=tb[:, hc, :], scalar=1.0,
                            in1=wb_ps[:],
                            op0=mybir.AluOpType.mult, op1=mybir.AluOpType.mult,
                            accum_out=part_ts[b][:, hc, t : t + 1],
                        )
                    tfs.pop(k, None)
                    tanhs.pop(k, None)
                    if t == nt - 1:
                        endb(b)

            # pipeline: loads/casts run PL tiles ahead; energy group g is
            # emitted after the P-matmuls of group g+1; ctx blocks trail one
            # tile per iteration so no engine stalls behind them.
            ngroups = total // G
            PL = 4
            for k in range(total + PL):
                if k < total:
                    loadcast(k)
                kp = k - PL
                if kp >= 0:
                    pmmtanh(kp)
                    do_pending(1)
                    if kp % G == G - 1 and kp // G >= 1:
                        process_energy(kp // G - 1)
            process_energy(ngroups - 1)
            do_pending(len(pending) + G)

    return nc


_CACHE = {}


def _get_nc(s_len):
    if s_len not in _CACHE:
        nc = bacc.Bacc("TRN2", target_bir_lowering=False, debug=False)
        build(nc, s_len)
        nc.compile()
        _CACHE[s_len] = nc
    return _CACHE[s_len]


def _prep_inputs(decoder_hidden, encoder_outputs, W1, W2, v):
    """Host-side shard: batch across 8 cores; encT layout (b, h, s) per core."""
    s_len = encoder_outputs.shape[0]
    w1t = np.ascontiguousarray(np.asarray(W1, dtype=np.float32).T)
    w2t = np.ascontiguousarray(np.asarray(W2, dtype=np.float32).T)
    v2d = np.ascontiguousarray(np.asarray(v, dtype=np.float32).reshape(1, H))
    enc = np.asarray(encoder_outputs, dtype=np.float32)
    dec = np.asarray(decoder_hidden, dtype=np.float32)
    in_maps = []
    for c in range(8):
        bsl = slice(c * NB, (c + 1) * NB)
        enc_c = np.ascontiguousarray(enc[:, bsl, :].transpose(1, 2, 0))
        dect_c = np.ascontiguousarray(dec[bsl, :].T)
        in_maps.append(
            {"enc": enc_c, "dect": dect_c, "w1t": w1t, "w2t": w2t, "v2d": v2d}
        )
    return in_maps, s_len


def kernel(decoder_hidden, encoder_outputs, W1, W2, v):
    global LAST_RESULTS
    in_maps, s_len = _prep_inputs(decoder_hidden, encoder_outputs, W1, W2, v)
    nc = _get_nc(s_len)
    res = run_bass_kernel_spmd(nc, in_maps, core_ids=list(range(8)), trace=TRACE)
    LAST_RESULTS = res
    B = 8 * NB
    context = np.empty((B, H), dtype=np.float32)
    attn = np.empty((B, s_len), dtype=np.float32)
    for c in range(8):
        bsl = slice(c * NB, (c + 1) * NB)
        context[bsl] = res.results[c]["ctx_out"]
        attn[bsl] = res.results[c]["attn_out"]
    return (context, attn)


# revision 28
# speedup vs baseline: 1.5617x; 1.0263x over previous
"""Bahdanau attention on 8 Trainium2 NeuronCores (Bass/Tile).

Problem (per reference):
  decoder_hidden (64, 512) f32, encoder_outputs (4096, 64, 512) f32,
  W1 (512,512), W2 (512,512), v (512,)
  dec_proj = decoder_hidden @ W1.T                       (B, H)
  enc_proj = einsum('bsh,gh->bsg', enc, W2)              (B, S, H)
  energy   = tanh(dec_proj[:,None,:] + enc_proj) @ v     (B, S)
  attn     = softmax(energy, axis=1)                     (B, S)
  context  = einsum('bs,bsh->bh', attn, enc)             (B, H)
  returns (context, attn)

Sharding: batch (64) split across 8 cores -> 8 batches/core; W1/W2/v
replicated. encoder_outputs is resharded host-side to (b, h, s) layout per
core so the contraction dim h lands on SBUF partitions; the kernel makes a
single pass over the 64 MB/core stream.

Per 512-column s-tile (one DMA of [128p, 4hc, 512s] f32, 2 KB rows):
  cast f32->bf16 (DVE)
  PT[g,s] = W2T-chunk.T @ encT-chunk   16 matmuls into one 4-bank PSUM tile
  tanh(PT + dec_projT[g,b])            4 ACT ops, bias folded per-partition
  energy = v.T @ tanhPT                4 matmuls -> psum [1, 512]
  w = exp(energy) (+ running sum via ACT accum), cast w bf16 (DVE)
  broadcast w across partitions (GPSIMD)
  ctx partials += encT * w             4 DVE scalar_tensor_tensor accums
End of batch: reduce partials, softmax-normalize, DMA outputs.
"""

import numpy as np
import ml_dtypes
from collections import deque

import concourse.bacc as bacc
import concourse.tile as tile
import concourse.mybir as mybir
import concourse.bass_isa as bass_isa
from concourse.bass_utils import run_bass_kernel_spmd

F32 = mybir.dt.float32
BF16 = mybir.dt.bfloat16
AF = mybir.ActivationFunctionType

NB = 8         # batches per core
H = 512
P = 128        # partitions
NCH = H // P   # h chunks (4)
TS = 512       # s columns per tile

TRACE = False
LAST_RESULTS = None


def build(nc, s_len):
    nt = s_len // TS  # s tiles per batch

    enc_d = nc.dram_tensor("enc", [NB, H, s_len], F32, kind="ExternalInput")
    dect_d = nc.dram_tensor("dect", [H, NB], F32, kind="ExternalInput")
    w1t_d = nc.dram_tensor("w1t", [H, H], F32, kind="ExternalInput")
    w2t_d = nc.dram_tensor("w2t", [H, H], F32, kind="ExternalInput")
    v_d = nc.dram_tensor("v2d", [1, H], F32, kind="ExternalInput")
    ctx_d = nc.dram_tensor("ctx_out", [NB, H], F32, kind="ExternalOutput")
    attn_d = nc.dram_tensor("attn_out", [NB, s_len], F32, kind="ExternalOutput")

    ones_d = nc.inline_tensor(np.ones((P, P), dtype=ml_dtypes.bfloat16), name="onespp")
    nt_ = s_len // TS
    G_ = min(4, nt_)
    mask_np = np.zeros((P, 1), dtype=np.float32)
    for j in range(G_):
        mask_np[32 * j, 0] = 1.0
    mask_d = nc.inline_tensor(mask_np, name="maskg")

    # persistent SBUF
    w2t_bf = nc.alloc_sbuf_tensor("w2t_bf", [P, NCH, H], BF16)   # [h, hc, g]
    ones_sb = nc.alloc_sbuf_tensor("ones_sb", [P, P], BF16)
    mask_sb = nc.alloc_sbuf_tensor("mask_sb", [P, 1], F32)
    v_sb = nc.alloc_sbuf_tensor("v_sb", [P, NCH], BF16)          # v chunks [g, gc]
    v32_sb = nc.alloc_sbuf_tensor("v32_sb", [P, NCH, 32], BF16)  # v padded for col-pack
    dpt_sb = nc.alloc_sbuf_tensor("dpt_sb", [P, NCH, NB], F32)   # dec_projT [g, gc, b]

    with tile.TileContext(nc) as tc:
        # ---------------- prologue ----------------
        with (
            tc.tile_pool(name="pro", bufs=1) as pro,
            tc.tile_pool(name="prop", bufs=1, space="PSUM") as prop,
        ):
            w2t_f = pro.tile([P, NCH, H], F32)
            nc.sync.dma_start(out=w2t_f[:], in_=w2t_d.ap().rearrange("(c p) g -> p c g", p=P))
            nc.scalar.copy(w2t_bf[:], w2t_f[:])

            nc.sync.dma_start(out=ones_sb[:], in_=ones_d[:])
            nc.sync.dma_start(out=mask_sb[:], in_=mask_d[:])

            v_f = pro.tile([P, NCH], F32)
            # v[g] -> [g % 128, g // 128]
            nc.sync.dma_start(out=v_f[:], in_=v_d.ap().rearrange("o (c p) -> p (o c)", p=P))
            nc.vector.tensor_copy(v_sb[:], v_f[:])
            nc.vector.memset(v32_sb[:], 0.0)
            for gc in range(NCH):
                nc.vector.tensor_copy(v32_sb[:, gc, 0:1], v_sb[:, gc : gc + 1])

            w1t_f = pro.tile([P, NCH, H], F32)
            nc.sync.dma_start(out=w1t_f[:], in_=w1t_d.ap().rearrange("(c p) g -> p c g", p=P))
            w1t_bf = pro.tile([P, NCH, H], BF16)
            nc.scalar.copy(w1t_bf[:], w1t_f[:])

            dect_f = pro.tile([P, NCH, NB], F32)
            nc.sync.dma_start(out=dect_f[:], in_=dect_d.ap().rearrange("(c p) b -> p c b", p=P))
            dect_bf = pro.tile([P, NCH, NB], BF16)
            nc.scalar.copy(dect_bf[:], dect_f[:])

            # dec_projT[g, b] = sum_h2 W1[g, h2] dec[b, h2]
            dp_ps = prop.tile([P, NCH, NB], F32)
            for gc in range(NCH):
                for hc in range(NCH):
                    nc.tensor.matmul(
                        dp_ps[:, gc, :],
                        w1t_bf[:, hc, gc * P : (gc + 1) * P],
                        dect_bf[:, hc, :],
                        start=(hc == 0), stop=(hc == NCH - 1),
                    )
            nc.scalar.copy(dpt_sb[:], dp_ps[:])

        # ---------------- main loop (software-pipelined emission) ----------------
        enc_r = [enc_d[b].rearrange("(c p) s -> p c s", p=P) for b in range(NB)]
        G = min(4, nt)  # energy col-pack group size
        assert nt % G == 0
        total = NB * nt

        with (
            tc.tile_pool(name="pio", bufs=8) as pio,
            tc.tile_pool(name="pbf", bufs=14) as pbf,
            tc.tile_pool(name="ptan", bufs=10) as ptan,
            tc.tile_pool(name="pw4", bufs=4) as pw4,
            tc.tile_pool(name="pacc", bufs=4) as pacc,
            tc.tile_pool(name="ppart", bufs=2) as ppart,
            tc.tile_pool(name="psml", bufs=2) as psml,
            tc.tile_pool(name="ppP", bufs=1, space="PSUM") as ppP,
            tc.tile_pool(name="ppP1", bufs=1, space="PSUM") as ppP1,
            tc.tile_pool(name="ppP2", bufs=1, space="PSUM") as ppP2,
            tc.tile_pool(name="ppP3", bufs=1, space="PSUM") as ppP3,
            tc.tile_pool(name="ppe", bufs=1, space="PSUM") as ppe,
            tc.tile_pool(name="ppsum", bufs=1, space="PSUM") as ppsum,
            tc.tile_pool(name="ppb", bufs=2, space="PSUM") as ppb,
        ):
            pending = deque()
            tfs = {}      # k -> f32 enc tile
            tbs = {}      # k -> bf16 enc tile
            tanhs = {}    # k -> tanh tile
            part_ts = {}  # b -> ctx partials
            w4s = {}      # group -> exp(energy) rows [128, TS] (rows 32j)
            accws = {}    # group -> per-row sums [128, 1] (rows 32j)

            def loadcast(k):
                """DMA + cast for tile k (keeps DVE casts ahead of ctx blocks)."""
                b, t = divmod(k, nt)
                if t == 0:
                    part_ts[b] = ppart.tile([P, NCH, nt], F32, tag="part", name="part")
                tf = pio.tile([P, NCH, TS], F32)
                nc.sync.dma_start(out=tf[:], in_=enc_r[b][:, :, t * TS : (t + 1) * TS])
                tb = pbf.tile([P, NCH, TS], BF16)
                nc.vector.tensor_copy(tb[:], tf[:])
                tfs[k] = tf
                tbs[k] = tb

            def pmmtanh(k):
                b, t = divmod(k, nt)
                tb = tbs[k]
                tanh_t = ptan.tile([P, NCH, TS], BF16)
                for gc in range(NCH):
                    pool = (ppP, ppP1, ppP2, ppP3)[gc]
                    pt_ps = pool.tile([P, TS], F32, tag="pt", name="pt")
                    for hc in range(NCH):
                        nc.tensor.matmul(
                            pt_ps[:],
                            w2t_bf[:, hc, gc * P : (gc + 1) * P],
                            tb[:, hc, :],
                            start=(hc == 0), stop=(hc == NCH - 1),
                        )
                    nc.scalar.activation(
                        tanh_t[:, gc, :], pt_ps[:], AF.Tanh,
                        bias=dpt_sb[:, gc, b : b + 1],
                    )
                tanhs[k] = tanh_t

            def process_energy(g):
                """Col-packed energy matmuls + one packed exp + PE broadcast +
                ctx accumulation for tiles Gg..Gg+G-1 (all same batch)."""
                b = (g * G) // nt
                e4 = ppe.tile([P, TS], F32, tag="e4", name="e4")
                for gc in range(NCH):
                    for j in range(G):
                        k = g * G + j
                        nc.tensor.matmul(
                            e4[32 * j : 32 * j + 32, :],
                            v32_sb[:, gc, :], tanhs[k][:, gc, :],
                            start=(gc == 0), stop=(gc == NCH - 1),
                            tile_position=(0, 32 * j),
                            skip_group_check=True,
                        )
                # one exp for the whole group; per-partition accum gives the
                # softmax partial sums on rows 32j for free
                w4 = pw4.tile([P, TS], F32, tag="w4", name="w4")
                accw = pacc.tile([P, 1], F32, tag="accw", name="accw")
                GG = 32 * G
                nc.scalar.activation(w4[0:GG, :], e4[0:GG, :], AF.Exp,
                                     accum_out=accw[0:GG, :])
                w4b = pw4.tile([P, TS], BF16, tag="w4b", name="w4b")
                nc.scalar.copy(w4b[0:GG, :], w4[0:GG, :])
                w4s[g] = w4
                accws[g] = accw

                for j in range(G):
                    pending.append((g * G + j, w4b, j))

            def endb(b):
                part_t = part_ts.pop(b)
                gpb = nt // G  # groups per batch
                g0 = b * gpb
                GG = 32 * G
                acc = accws[g0]
                if gpb > 1:
                    acc_t = psml.tile([P, 1], F32, tag="acct")
                    nc.vector.tensor_tensor(
                        out=acc_t[0:GG, :], in0=accws[g0][0:GG, :],
                        in1=accws[g0 + 1][0:GG, :], op=mybir.AluOpType.add)
                    for g in range(2, gpb):
                        nc.vector.tensor_tensor(
                            out=acc_t[0:GG, :], in0=acc_t[0:GG, :],
                            in1=accws[g0 + g][0:GG, :], op=mybir.AluOpType.add)
                    acc = acc_t
                # sum of rows {32j} only via masked fp32 matmul
                sum_ps = ppsum.tile([1, 1], F32, tag="sum", name="sum")
                nc.tensor.matmul(sum_ps[:], acc[0:GG, :], mask_sb[0:GG, :],
                                 start=True, stop=True)
                inv1 = psml.tile([1, 1], F32, tag="inv1")
                nc.vector.reciprocal(inv1[:], sum_ps[:])
                inv_b = psml.tile([P, 1], F32, tag="invb")
                nc.gpsimd.partition_broadcast(inv_b[:], inv1[:])

                for g in range(g0, g0 + gpb):
                    w4 = w4s.pop(g)
                    nc.vector.tensor_scalar(
                        out=w4[0:GG, :], in0=w4[0:GG, :],
                        scalar1=inv_b[0:GG, :], scalar2=None,
                        op0=mybir.AluOpType.mult,
                    )
                    for j in range(G):
                        s0 = (g * G + j) % nt * TS
                        nc.sync.dma_start(
                            out=attn_d[b : b + 1, s0 : s0 + TS],
                            in_=w4[32 * j : 32 * j + 1, :])
                    accws.pop(g, None)

                ctx_red = psml.tile([P, NCH], F32, tag="ctxred")
                nc.vector.reduce_sum(ctx_red[:], part_t[:], axis=mybir.AxisListType.X)
                ctx_sb = psml.tile([P, NCH], F32, tag="ctxo")
                nc.vector.tensor_scalar(
                    out=ctx_sb[:], in0=ctx_red[:], scalar1=inv_b[:],
                    scalar2=None, op0=mybir.AluOpType.mult,
                )
                nc.sync.dma_start(out=ctx_d[b].rearrange("(c p) -> p c", p=P), in_=ctx_sb[:])

            def do_pending(n):
                """Emit wb broadcast-MM + ctx stt block for up to n pending
                tiles (spread across iterations so PE never stalls on the
                wb PSUM WAR against in-flight ctx blocks)."""
                for _ in range(min(n, len(pending))):
                    k, w4b, j = pending.popleft()
                    b, t = divmod(k, nt)
                    wb_ps = ppb.tile([P, TS], F32, tag="wb", name="wb")
                    nc.tensor.matmul(
                        wb_ps[:], ones_sb[32 * j : 32 * j + 1, :],
                        w4b[32 * j : 32 * j + 1, :],
                        start=True, stop=True, tile_position=(32 * j, 0),
                    )
                    tb = tbs.pop(k)
                    for hc in range(NCH):
                        nc.vector.scalar_tensor_tensor(
                            out=tb[:, hc, :], in0=tb[:, hc, :], scalar=1.0,
                            in1=wb_ps[:],
                            op0=mybir.AluOpType.mult, op1=mybir.AluOpType.mult,
                            accum_out=part_ts[b][:, hc, t : t + 1],
                        )
                    tfs.pop(k, None)
                    tanhs.pop(k, None)
                    if t == nt - 1:
                        endb(b)

            # pipeline: loads/casts run PL tiles ahead; energy group g is
            # emitted after the P-matmuls of group g+1; ctx blocks trail one
            # tile per iteration so no engine stalls behind them.
            ngroups = total // G
            PL = 4
            for k in range(total + PL):
                if k < total:
                    loadcast(k)
                kp = k - PL
                if kp >= 0:
                    pmmtanh(kp)
                    do_pending(1)
                    if kp % G == G - 1 and kp // G >= 1:
                        process_energy(kp // G - 1)
            process_energy(ngroups - 1)
            do_pending(len(pending) + G)

    return nc


_CACHE = {}


def _get_nc(s_len):
    if s_len not in _CACHE:
        nc = bacc.Bacc("TRN2", target_bir_lowering=False, debug=False)
        build(nc, s_len)
        nc.compile()
        _CACHE[s_len] = nc
    return _CACHE[s_len]


def _prep_inputs(decoder_hidden, encoder_outputs, W1, W2, v):
    """Host-side shard: batch across 8 cores; encT layout (b, h, s) per core."""
    s_len = encoder_outputs.shape[0]
    w1t = np.ascontiguousarray(np.asarray(W1, dtype=np.float32).T)
    w2t = np.ascontiguousarray(np.asarray(W2, dtype=np.float32).T)
    v2d = np.ascontiguousarray(np.asarray(v, dtype=np.float32).reshape(1, H))
    enc = np.asarray(encoder_outputs, dtype=np.float32)
    dec = np.asarray(decoder_hidden, dtype=np.float32)
    in_maps = []
    for c in range(8):
        bsl = slice(c * NB, (c + 1) * NB)
        enc_c = np.ascontiguousarray(enc[:, bsl, :].transpose(1, 2, 0))
        dect_c = np.ascontiguousarray(dec[bsl, :].T)
        in_maps.append(
            {"enc": enc_c, "dect": dect_c, "w1t": w1t, "w2t": w2t, "v2d": v2d}
        )
    return in_maps, s_len


def kernel(decoder_hidden, encoder_outputs, W1, W2, v):
    global LAST_RESULTS
    in_maps, s_len = _prep_inputs(decoder_hidden, encoder_outputs, W1, W2, v)
    nc = _get_nc(s_len)
    res = run_bass_kernel_spmd(nc, in_maps, core_ids=list(range(8)), trace=TRACE)
    LAST_RESULTS = res
    B = 8 * NB
    context = np.empty((B, H), dtype=np.float32)
    attn = np.empty((B, s_len), dtype=np.float32)
    for c in range(8):
        bsl = slice(c * NB, (c + 1) * NB)
        context[bsl] = res.results[c]["ctx_out"]
        attn[bsl] = res.results[c]["attn_out"]
    return (context, attn)


# revision 29
# speedup vs baseline: 1.6180x; 1.0360x over previous
"""Bahdanau attention on 8 Trainium2 NeuronCores (Bass/Tile).

Problem (per reference):
  decoder_hidden (64, 512) f32, encoder_outputs (4096, 64, 512) f32,
  W1 (512,512), W2 (512,512), v (512,)
  dec_proj = decoder_hidden @ W1.T                       (B, H)
  enc_proj = einsum('bsh,gh->bsg', enc, W2)              (B, S, H)
  energy   = tanh(dec_proj[:,None,:] + enc_proj) @ v     (B, S)
  attn     = softmax(energy, axis=1)                     (B, S)
  context  = einsum('bs,bsh->bh', attn, enc)             (B, H)
  returns (context, attn)

Sharding: batch (64) split across 8 cores -> 8 batches/core; W1/W2/v
replicated. encoder_outputs is resharded host-side to (b, h, s) layout per
core so the contraction dim h lands on SBUF partitions; the kernel makes a
single pass over the 64 MB/core stream.

Per 512-column s-tile (one DMA of [128p, 4hc, 512s] f32, 2 KB rows):
  cast f32->bf16 (DVE)
  PT[g,s] = W2T-chunk.T @ encT-chunk   16 matmuls, one 1-bank PSUM tile per gc
  tanh(PT + dec_projT[g,b])            4 ACT ops, bias folded per-partition
Per group of 4 tiles (software-pipelined one group behind the P-matmuls):
  energy = v.T @ tanhPT                col-packed matmuls (tile_position) into
                                       rows {32j} of one PSUM bank
  w = exp(energy)                      one packed ACT op; per-partition accum
                                       gives softmax partial sums for free
  broadcast w: ones-column outer product matmul per tile (PE)
  ctx partials += encT * w             4 DVE scalar_tensor_tensor accums/tile
End of batch: masked-matmul row sum, reciprocal, normalize, DMA outputs.
"""

import numpy as np
import ml_dtypes
from collections import deque

import concourse.bacc as bacc
import concourse.tile as tile
import concourse.mybir as mybir
import concourse.bass_isa as bass_isa
from concourse.bass_utils import run_bass_kernel_spmd

F32 = mybir.dt.float32
BF16 = mybir.dt.bfloat16
AF = mybir.ActivationFunctionType

NB = 8         # batches per core
H = 512
P = 128        # partitions
NCH = H // P   # h chunks (4)
TS = 512       # s columns per tile

TRACE = False
LAST_RESULTS = None


def build(nc, s_len):
    nt = s_len // TS  # s tiles per batch

    enc_d = nc.dram_tensor("enc", [NB, H, s_len], F32, kind="ExternalInput")
    dect_d = nc.dram_tensor("dect", [H, NB], F32, kind="ExternalInput")
    w1t_d = nc.dram_tensor("w1t", [H, H], F32, kind="ExternalInput")
    w2t_d = nc.dram_tensor("w2t", [H, H], F32, kind="ExternalInput")
    v_d = nc.dram_tensor("v2d", [1, H], F32, kind="ExternalInput")
    ctx_d = nc.dram_tensor("ctx_out", [NB, H], F32, kind="ExternalOutput")
    attn_d = nc.dram_tensor("attn_out", [NB, s_len], F32, kind="ExternalOutput")

    ones_d = nc.inline_tensor(np.ones((P, P), dtype=ml_dtypes.bfloat16), name="onespp")
    nt_ = s_len // TS
    G_ = min(4, nt_)
    mask_np = np.zeros((P, 1), dtype=np.float32)
    for j in range(G_):
        mask_np[32 * j, 0] = 1.0
    mask_d = nc.inline_tensor(mask_np, name="maskg")

    # persistent SBUF
    w2t_bf = nc.alloc_sbuf_tensor("w2t_bf", [P, NCH, H], BF16)   # [h, hc, g]
    ones_sb = nc.alloc_sbuf_tensor("ones_sb", [P, P], BF16)
    mask_sb = nc.alloc_sbuf_tensor("mask_sb", [P, 1], F32)
    v_sb = nc.alloc_sbuf_tensor("v_sb", [P, NCH], BF16)          # v chunks [g, gc]
    v32_sb = nc.alloc_sbuf_tensor("v32_sb", [P, NCH, 32], BF16)  # v padded for col-pack
    dpt_sb = nc.alloc_sbuf_tensor("dpt_sb", [P, NCH, NB], F32)   # dec_projT [g, gc, b]

    with tile.TileContext(nc) as tc:
        # ---------------- prologue ----------------
        with (
            tc.tile_pool(name="pro", bufs=1) as pro,
            tc.tile_pool(name="prop", bufs=1, space="PSUM") as prop,
        ):
            w2t_f = pro.tile([P, NCH, H], F32)
            nc.sync.dma_start(out=w2t_f[:], in_=w2t_d.ap().rearrange("(c p) g -> p c g", p=P))
            nc.scalar.copy(w2t_bf[:], w2t_f[:])

            nc.sync.dma_start(out=ones_sb[:], in_=ones_d[:])
            nc.sync.dma_start(out=mask_sb[:], in_=mask_d[:])

            v_f = pro.tile([P, NCH], F32)
            # v[g] -> [g % 128, g // 128]
            nc.sync.dma_start(out=v_f[:], in_=v_d.ap().rearrange("o (c p) -> p (o c)", p=P))
            nc.vector.tensor_copy(v_sb[:], v_f[:])
            nc.vector.memset(v32_sb[:], 0.0)
            for gc in range(NCH):
                nc.vector.tensor_copy(v32_sb[:, gc, 0:1], v_sb[:, gc : gc + 1])

            w1t_f = pro.tile([P, NCH, H], F32)
            nc.sync.dma_start(out=w1t_f[:], in_=w1t_d.ap().rearrange("(c p) g -> p c g", p=P))
            w1t_bf = pro.tile([P, NCH, H], BF16)
            nc.scalar.copy(w1t_bf[:], w1t_f[:])

            dect_f = pro.tile([P, NCH, NB], F32)
            nc.sync.dma_start(out=dect_f[:], in_=dect_d.ap().rearrange("(c p) b -> p c b", p=P))
            dect_bf = pro.tile([P, NCH, NB], BF16)
            nc.scalar.copy(dect_bf[:], dect_f[:])

            # dec_projT[g, b] = sum_h2 W1[g, h2] dec[b, h2]
            dp_ps = prop.tile([P, NCH, NB], F32)
            for gc in range(NCH):
                for hc in range(NCH):
                    nc.tensor.matmul(
                        dp_ps[:, gc, :],
                        w1t_bf[:, hc, gc * P : (gc + 1) * P],
                        dect_bf[:, hc, :],
                        start=(hc == 0), stop=(hc == NCH - 1),
                    )
            nc.scalar.copy(dpt_sb[:], dp_ps[:])

        # ---------------- main loop (software-pipelined emission) ----------------
        enc_r = [enc_d[b].rearrange("(c p) s -> p c s", p=P) for b in range(NB)]
        G = min(4, nt)  # energy col-pack group size
        assert nt % G == 0
        total = NB * nt

        with (
            tc.tile_pool(name="pio", bufs=8) as pio,
            tc.tile_pool(name="pbf", bufs=14) as pbf,
            tc.tile_pool(name="ptan", bufs=10) as ptan,
            tc.tile_pool(name="pw4", bufs=4) as pw4,
            tc.tile_pool(name="pacc", bufs=4) as pacc,
            tc.tile_pool(name="ppart", bufs=2) as ppart,
            tc.tile_pool(name="psml", bufs=2) as psml,
            tc.tile_pool(name="ppP", bufs=1, space="PSUM") as ppP,
            tc.tile_pool(name="ppP1", bufs=1, space="PSUM") as ppP1,
            tc.tile_pool(name="ppP2", bufs=1, space="PSUM") as ppP2,
            tc.tile_pool(name="ppP3", bufs=1, space="PSUM") as ppP3,
            tc.tile_pool(name="ppe", bufs=1, space="PSUM") as ppe,
            tc.tile_pool(name="ppsum", bufs=1, space="PSUM") as ppsum,
            tc.tile_pool(name="ppb", bufs=2, space="PSUM") as ppb,
        ):
            pending = deque()
            tfs = {}      # k -> f32 enc tile
            tbs = {}      # k -> bf16 enc tile
            tanhs = {}    # k -> tanh tile
            part_ts = {}  # b -> ctx partials
            w4s = {}      # group -> exp(energy) rows [128, TS] (rows 32j)
            accws = {}    # group -> per-row sums [128, 1] (rows 32j)

            def loadcast(k):
                """DMA + cast for tile k (keeps DVE casts ahead of ctx blocks)."""
                b, t = divmod(k, nt)
                if t == 0:
                    part_ts[b] = ppart.tile([P, NCH, nt], F32, tag="part", name="part")
                tf = pio.tile([P, NCH, TS], F32)
                nc.sync.dma_start(out=tf[:], in_=enc_r[b][:, :, t * TS : (t + 1) * TS])
                tb = pbf.tile([P, NCH, TS], BF16)
                nc.vector.tensor_copy(tb[:], tf[:])
                tfs[k] = tf
                tbs[k] = tb

            def pmmtanh(k):
                b, t = divmod(k, nt)
                tb = tbs[k]
                tanh_t = ptan.tile([P, NCH, TS], BF16)
                for gc in range(NCH):
                    pool = (ppP, ppP1, ppP2, ppP3)[gc]
                    pt_ps = pool.tile([P, TS], F32, tag="pt", name="pt")
                    for hc in range(NCH):
                        nc.tensor.matmul(
                            pt_ps[:],
                            w2t_bf[:, hc, gc * P : (gc + 1) * P],
                            tb[:, hc, :],
                            start=(hc == 0), stop=(hc == NCH - 1),
                        )
                    nc.scalar.activation(
                        tanh_t[:, gc, :], pt_ps[:], AF.Tanh,
                        bias=dpt_sb[:, gc, b : b + 1],
                    )
                tanhs[k] = tanh_t

            def process_energy(g):
                """Col-packed energy matmuls + one packed exp + PE broadcast +
                ctx accumulation for tiles Gg..Gg+G-1 (all same batch)."""
                b = (g * G) // nt
                e4 = ppe.tile([P, TS], F32, tag="e4", name="e4")
                for gc in range(NCH):
                    for j in range(G):
                        k = g * G + j
                        nc.tensor.matmul(
                            e4[32 * j : 32 * j + 32, :],
                            v32_sb[:, gc, :], tanhs[k][:, gc, :],
                            start=(gc == 0), stop=(gc == NCH - 1),
                            tile_position=(0, 32 * j),
                            skip_group_check=True,
                        )
                # one exp for the whole group; per-partition accum gives the
                # softmax partial sums on rows 32j for free
                w4 = pw4.tile([P, TS], F32, tag="w4", name="w4")
                accw = pacc.tile([P, 1], F32, tag="accw", name="accw")
                GG = 32 * G
                nc.scalar.activation(w4[0:GG, :], e4[0:GG, :], AF.Exp,
                                     accum_out=accw[0:GG, :])
                w4b = pw4.tile([P, TS], BF16, tag="w4b", name="w4b")
                nc.scalar.copy(w4b[0:GG, :], w4[0:GG, :])
                w4s[g] = w4
                accws[g] = accw

                for j in range(G):
                    pending.append((g * G + j, w4b, j))

            def endb(b):
                part_t = part_ts.pop(b)
                gpb = nt // G  # groups per batch
                g0 = b * gpb
                GG = 32 * G
                acc = accws[g0]
                if gpb > 1:
                    acc_t = psml.tile([P, 1], F32, tag="acct")
                    nc.vector.tensor_tensor(
                        out=acc_t[0:GG, :], in0=accws[g0][0:GG, :],
                        in1=accws[g0 + 1][0:GG, :], op=mybir.AluOpType.add)
                    for g in range(2, gpb):
                        nc.vector.tensor_tensor(
                            out=acc_t[0:GG, :], in0=acc_t[0:GG, :],
                            in1=accws[g0 + g][0:GG, :], op=mybir.AluOpType.add)
                    acc = acc_t
                # sum of rows {32j} only via masked fp32 matmul
                sum_ps = ppsum.tile([1, 1], F32, tag="sum", name="sum")
                nc.tensor.matmul(sum_ps[:], acc[0:GG, :], mask_sb[0:GG, :],
                                 start=True, stop=True)
                inv1 = psml.tile([1, 1], F32, tag="inv1")
                nc.vector.reciprocal(inv1[:], sum_ps[:])
                inv_b = psml.tile([P, 1], F32, tag="invb")
                nc.gpsimd.partition_broadcast(inv_b[:], inv1[:])

                for g in range(g0, g0 + gpb):
                    w4 = w4s.pop(g)
                    nc.vector.tensor_scalar(
                        out=w4[0:GG, :], in0=w4[0:GG, :],
                        scalar1=inv_b[0:GG, :], scalar2=None,
                        op0=mybir.AluOpType.mult,
                    )
                    for j in range(G):
                        s0 = (g * G + j) % nt * TS
                        nc.sync.dma_start(
                            out=attn_d[b : b + 1, s0 : s0 + TS],
                            in_=w4[32 * j : 32 * j + 1, :])
                    accws.pop(g, None)

                ctx_red = psml.tile([P, NCH], F32, tag="ctxred")
                nc.vector.reduce_sum(ctx_red[:], part_t[:], axis=mybir.AxisListType.X)
                ctx_sb = psml.tile([P, NCH], F32, tag="ctxo")
                nc.vector.tensor_scalar(
                    out=ctx_sb[:], in0=ctx_red[:], scalar1=inv_b[:],
                    scalar2=None, op0=mybir.AluOpType.mult,
                )
                nc.sync.dma_start(out=ctx_d[b].rearrange("(c p) -> p c", p=P), in_=ctx_sb[:])

            def do_pending(n):
                """Emit wb broadcast-MM + ctx stt block for up to n pending
                tiles (spread across iterations so PE never stalls on the
                wb PSUM WAR against in-flight ctx blocks)."""
                for _ in range(min(n, len(pending))):
                    k, w4b, j = pending.popleft()
                    b, t = divmod(k, nt)
                    wb_ps = ppb.tile([P, TS], F32, tag="wb", name="wb")
                    nc.tensor.matmul(
                        wb_ps[:], ones_sb[32 * j : 32 * j + 1, :],
                        w4b[32 * j : 32 * j + 1, :],
                        start=True, stop=True, tile_position=(32 * j, 0),
                    )
                    tb = tbs.pop(k)
                    for hc in range(NCH):
                        nc.vector.scalar_tensor_tensor(
                            out=tb[:, hc, :], in0=tb[:, hc, :], scalar=1.0,
                            in1=wb_ps[:],
                            op0=mybir.AluOpType.mult, op1=mybir.AluOpType.mult,
                            accum_out=part_ts[b][:, hc, t : t + 1],
                        )
                    tfs.pop(k, None)
                    tanhs.pop(k, None)
                    if t == nt - 1:
                        endb(b)

            # pipeline: loads/casts run PL tiles ahead; energy group g is
            # emitted after the P-matmuls of group g+1; ctx blocks trail one
            # tile per iteration so no engine stalls behind them.
            ngroups = total // G
            PL = 4
            for k in range(total + PL):
                if k < total:
                    loadcast(k)
                kp = k - PL
                if kp >= 0:
                    pmmtanh(kp)
                    do_pending(1)
                    if kp % G == G - 1 and kp // G >= 1:
                        process_energy(kp // G - 1)
            process_energy(ngroups - 1)
            do_pending(len(pending) + G)

    return nc


_CACHE = {}


def _get_nc(s_len):
    if s_len not in _CACHE:
        nc = bacc.Bacc("TRN2", target_bir_lowering=False, debug=False)
        build(nc, s_len)
        nc.compile()
        _CACHE[s_len] = nc
    return _CACHE[s_len]


def _prep_inputs(decoder_hidden, encoder_outputs, W1, W2, v):
    """Host-side shard: batch across 8 cores; encT layout (b, h, s) per core."""
    s_len = encoder_outputs.shape[0]
    w1t = np.ascontiguousarray(np.asarray(W1, dtype=np.float32).T)
    w2t = np.ascontiguousarray(np.asarray(W2, dtype=np.float32).T)
    v2d = np.ascontiguousarray(np.asarray(v, dtype=np.float32).reshape(1, H))
    enc = np.asarray(encoder_outputs, dtype=np.float32)
    dec = np.asarray(decoder_hidden, dtype=np.float32)
    in_maps = []
    for c in range(8):
        bsl = slice(c * NB, (c + 1) * NB)
        enc_c = np.ascontiguousarray(enc[:, bsl, :].transpose(1, 2, 0))
        dect_c = np.ascontiguousarray(dec[bsl, :].T)
        in_maps.append(
            {"enc": enc_c, "dect": dect_c, "w1t": w1t, "w2t": w2t, "v2d": v2d}
        )
    return in_maps, s_len


def kernel(decoder_hidden, encoder_outputs, W1, W2, v):
    global LAST_RESULTS
    in_maps, s_len = _prep_inputs(decoder_hidden, encoder_outputs, W1, W2, v)
    nc = _get_nc(s_len)
    res = run_bass_kernel_spmd(nc, in_maps, core_ids=list(range(8)), trace=TRACE)
    LAST_RESULTS = res
    B = 8 * NB
    context = np.empty((B, H), dtype=np.float32)
    attn = np.empty((B, s_len), dtype=np.float32)
    for c in range(8):
        bsl = slice(c * NB, (c + 1) * NB)
        context[bsl] = res.results[c]["ctx_out"]
        attn[bsl] = res.results[c]["attn_out"]
    return (context, attn)
